# revision 29
# baseline (speedup 1.0000x reference)
"""Trainium2 Bass kernel for nn_DecoderCell: embedding gather -> Bahdanau
attention -> single-step LSTM -> vocab softmax, distributed over 8 NeuronCores.

Sharding (hardcoded, per spec sharding_hint):
  - attention: batch-sharded (8 batches/core, enc_out slice 4MB/core)
  - LSTM: hidden-dim sharded (128 h-dims/core -> 4x128 gate columns/core)
  - fc/softmax: vocab-sharded (4000 vocab cols/core, Wfc slice 16MB/core)
  - collectives: AllGather(context rows), AllGather(h_new^T), AllReduce(softmax sum)

kernel(**inputs) takes FULL unsharded inputs, returns (probs, h_new, c_new).
"""
import numpy as np
import ml_dtypes

import concourse.bacc as bacc
import concourse.bass as bass
import concourse.tile as tile
from concourse import mybir
from concourse.bass_utils import run_bass_kernel_spmd
from concourse.masks import make_identity

F32 = mybir.dt.float32
F32R = mybir.dt.float32r
BF16 = mybir.dt.bfloat16
I32 = mybir.dt.int32
AF = mybir.ActivationFunctionType
ALU = mybir.AluOpType
AX = mybir.AxisListType

N_CORES = 8
B, S, VOCAB, EMB, UNITS = 64, 128, 32000, 256, 512
H = 2 * UNITS            # 1024
DENC = 2 * UNITS         # 1024
IN = EMB + DENC          # 1280
BL = B // N_CORES        # 8 local batches
HC = H // N_CORES        # 128 h-dims per core
VC = VOCAB // N_CORES    # 4000 vocab cols per core
NCH = 500                # fc column chunk (<=512 psum bank)
NNC = VC // NCH          # 8 fc chunks
KD = DENC // 128         # 8
KH = H // 128            # 8
KU = UNITS // 128        # 4
KE = EMB // 128          # 2
KIN = IN // 128          # 10
GRP = 2                  # attention batch groups
GB = BL // GRP           # 4 batches per group


def build_kernel(nc):
    enc = nc.dram_tensor("enc", [BL, S, DENC], BF16, kind="ExternalInput")
    enc_t = nc.dram_tensor("enc_t", [DENC, BL * S], BF16, kind="ExternalInput")
    h_t = nc.dram_tensor("hT", [H, B], F32R, kind="ExternalInput")
    hl_t = nc.dram_tensor("h_locT", [H, BL], F32R, kind="ExternalInput")
    token = nc.dram_tensor("token", [B, 1], I32, kind="ExternalInput")
    emb_table = nc.dram_tensor("emb_table", [VOCAB, EMB], F32, kind="ExternalInput")
    c_sl = nc.dram_tensor("c_sl", [B, HC], F32, kind="ExternalInput")
    w1 = nc.dram_tensor("W1", [DENC, UNITS], BF16, kind="ExternalInput")
    w2 = nc.dram_tensor("W2", [DENC, UNITS], F32R, kind="ExternalInput")
    v_in = nc.dram_tensor("V", [UNITS, 1], BF16, kind="ExternalInput")
    wx = nc.dram_tensor("Wx_sl", [IN, 4 * HC], F32R, kind="ExternalInput")
    wh = nc.dram_tensor("Wh_sl", [H, 4 * HC], F32R, kind="ExternalInput")
    b_sl = nc.dram_tensor("b_sl", [1, 4 * HC], F32R, kind="ExternalInput")
    wfc = nc.dram_tensor("Wfc_sl", [H, VC], BF16, kind="ExternalInput")
    bfc = nc.dram_tensor("bfc_sl", [1, VC], BF16, kind="ExternalInput")

    probs_out = nc.dram_tensor("probs", [B, VC], F32, kind="ExternalOutput")
    h_out = nc.dram_tensor("h_sl", [B, HC], F32, kind="ExternalOutput")
    c_out = nc.dram_tensor("c_sl_out", [B, HC], F32, kind="ExternalOutput")

    rg = [list(range(N_CORES))]

    with tile.TileContext(nc) as tc:
        with (
            tc.tile_pool(name="dram", bufs=1, space="DRAM") as dram,
            tc.tile_pool(name="const", bufs=1) as constp,
            tc.tile_pool(name="wt", bufs=1) as wtp,
            tc.tile_pool(name="small", bufs=1) as smallp,
            tc.tile_pool(name="attn", bufs=1) as attnp,
            tc.tile_pool(name="epool", bufs=5) as epool,
            tc.tile_pool(name="wfcp", bufs=4) as wfcp,
            tc.tile_pool(name="expp", bufs=1) as expp,
            tc.tile_pool(name="ptr", bufs=2, space="PSUM") as ptr,
            tc.tile_pool(name="pacc", bufs=1, space="PSUM") as pacc,
            tc.tile_pool(name="pa", bufs=3, space="PSUM") as pap,
            tc.tile_pool(name="prow", bufs=2, space="PSUM") as prow,
        ):
            # DRAM bounce buffers for collectives
            dum_in = dram.tile([1, 8], F32)
            dum_out = dram.tile([N_CORES, 8], F32)
            dum2_in = dram.tile([1, 8], F32)
            dum2_out = dram.tile([1, 8], F32)
            ctx_ag_in = dram.tile([BL, DENC], F32)
            ctx_ag_out = dram.tile([B, DENC], F32)
            hn_ag_in = dram.tile([HC, B], F32R)
            hn_ag_out = dram.tile([H, B], F32R)
            ar_half = [dram.tile([B, 1], F32, name=f"arh{i}") for i in range(2)]
            ar_half_out = [dram.tile([B, 1], F32, name=f"arho{i}") for i in range(2)]

            # warm up the collective path (absorbs the first-collective global
            # barrier + ncfw init while input DMAs stream)
            dum_sb = smallp.tile([1, 8], F32)
            nc.vector.memset(dum_sb[:], 0.0)
            nc.scalar.dma_start(out=dum_in[:], in_=dum_sb[:])
            nc.gpsimd.collective_compute(
                "AllGather", ALU.bypass, replica_groups=rg,
                ins=[dum_in[:].opt()], outs=[dum_out[:].opt()],
            )
            nc.scalar.dma_start(out=dum2_in[:], in_=dum_sb[:])
            nc.gpsimd.collective_compute(
                "AllReduce", ALU.add, replica_groups=rg,
                ins=[dum2_in[:].opt()], outs=[dum2_out[:].opt()],
            )

            # constants
            ident = constp.tile([128, 128], F32)
            make_identity(nc, ident[:])
            ones_f = constp.tile([1, B], F32)
            nc.vector.memset(ones_f[:], 1.0)
            ones_r = constp.tile([1, B], F32R)
            nc.vector.tensor_copy(ones_r[:], ones_f[:])
            ones_b = constp.tile([1, B], BF16)
            nc.vector.tensor_copy(ones_b[:], ones_f[:])

            # ---------------- phase 0: loads ----------------
            tok = smallp.tile([B, 1], I32)
            nc.scalar.dma_start(out=tok[:], in_=token[:])
            emb_sb = smallp.tile([B, EMB], F32)
            nc.gpsimd.indirect_dma_start(
                out=emb_sb[:],
                out_offset=None,
                in_=emb_table[:],
                in_offset=bass.IndirectOffsetOnAxis(ap=tok[:, :1], axis=0),
            )
            c_sb = smallp.tile([B, HC], F32)
            nc.scalar.dma_start(out=c_sb[:], in_=c_sl[:])

            w1all = wtp.tile([128, KD * UNITS], BF16)
            nc.sync.dma_start(
                out=w1all[:].rearrange("p (k u) -> p k u", k=KD),
                in_=w1[:, :].rearrange("(k p) u -> p k u", p=128),
            )
            w2all = wtp.tile([128, KD * UNITS], F32R)
            nc.scalar.dma_start(
                out=w2all[:].rearrange("p (k u) -> p k u", k=KD),
                in_=w2[:, :].rearrange("(k p) u -> p k u", p=128),
            )
            vt = wtp.tile([128, KU], BF16)
            nc.scalar.dma_start(
                out=vt[:].rearrange("p (k o) -> p k o", k=KU),
                in_=v_in[:, :].rearrange("(k p) o -> p k o", p=128),
            )
            wxall = wtp.tile([128, KIN * 4 * HC], F32R)
            nc.sync.dma_start(
                out=wxall[:].rearrange("p (k u) -> p k u", k=KIN),
                in_=wx[:, :].rearrange("(k p) u -> p k u", p=128),
            )
            whall = wtp.tile([128, KH * 4 * HC], F32R)
            nc.scalar.dma_start(
                out=whall[:].rearrange("p (k u) -> p k u", k=KH),
                in_=wh[:, :].rearrange("(k p) u -> p k u", p=128),
            )
            b_sb = smallp.tile([1, 4 * HC], F32R)
            nc.scalar.dma_start(out=b_sb[:], in_=b_sl[:])

            # ---------------- phase T: input transposes ----------------
            def transpose_to(dst_ap, src_ap, p_in, p_out):
                tr = ptr.tile([128, 128], F32, tag="tr", bufs=2, name="tr")
                nc.tensor.transpose(tr[:p_out, :p_in], src_ap, ident[:p_in, :p_in])
                nc.vector.tensor_copy(dst_ap, tr[:p_out, :p_in])

            htall = smallp.tile([128, KH * B], F32R)
            nc.sync.dma_start(
                out=htall[:].rearrange("p (k b) -> p k b", k=KH),
                in_=h_t[:, :].rearrange("(k p) b -> p k b", p=128),
            )
            htk = [htall[:, B * k : B * (k + 1)] for k in range(KH)]
            hlall = smallp.tile([128, KH * BL], F32R)
            nc.scalar.dma_start(
                out=hlall[:].rearrange("p (k b) -> p k b", k=KH),
                in_=hl_t[:, :].rearrange("(k p) b -> p k b", p=128),
            )
            hlk = [hlall[:, BL * k : BL * (k + 1)] for k in range(KH)]
            xtk = []  # x^T chunks [128, 64] f32r: emb (2) then ctx (8)
            for k in range(KE):
                d = smallp.tile([128, B], F32R, name=f"xtk{k}")
                transpose_to(d[:], emb_sb[:, 128 * k : 128 * (k + 1)], B, 128)
                xtk.append(d)

            # ---------------- phase Q: q = h_loc @ W2 (natural layout) -------
            qp = pacc.tile([BL, UNITS], F32, tag="acc", name="qp")
            for k in range(KH):
                nc.tensor.matmul(
                    qp[:],
                    hlk[k],
                    w2all[:, UNITS * k : UNITS * (k + 1)],
                    start=(k == 0),
                    stop=(k == KH - 1),
                )
            q_sb = smallp.tile([BL, UNITS], F32)
            nc.vector.tensor_copy(q_sb[:], qp[:])
            qt = smallp.tile([128, KU * BL], F32)  # q^T: [128(u), 8] per u-chunk
            for u in range(KU):
                transpose_to(
                    qt[:, BL * u : BL * (u + 1)],
                    q_sb[:, 128 * u : 128 * (u + 1)],
                    BL,
                    128,
                )

            # ---------------- phase A: attention (bf16, DMA-transposed E) ----
            esums = smallp.tile([1, BL], F32)
            rsum = smallp.tile([1, BL], F32)
            wc8 = smallp.tile([128, BL], BF16)

            etall = attnp.tile([128, KD * BL * S], BF16, tag="big", bufs=1)
            nc.sync.dma_start(
                out=etall[:].rearrange("p (k c) -> p k c", k=KD),
                in_=enc_t[:, :].rearrange("(k p) c -> p k c", p=128),
            )
            etg = [
                [etall[:, 1024 * d + 512 * hf : 1024 * d + 512 * (hf + 1)] for hf in range(2)]
                for d in range(KD)
            ]
            eall = epool.tile([S, BL * DENC], BF16, tag="ebig", bufs=1)
            nc.sync.dma_start(
                out=eall[:].rearrange("s (b d) -> s b d", b=BL),
                in_=enc[:, :, :].rearrange("b s d -> s b d"),
            )
            e_tiles = [eall[:, DENC * b_ : DENC * (b_ + 1)] for b_ in range(BL)]

            for hf in range(2):
                tt = []
                for u in range(KU):
                    pa = pap.tile([128, 512], F32, tag="pa", name=f"pa{u}_{hf}")
                    for k in range(KD):
                        nc.tensor.matmul(
                            pa[:],
                            w1all[:, UNITS * k + 128 * u : UNITS * k + 128 * (u + 1)],
                            etg[k][hf][:],
                            start=(k == 0),
                            stop=(k == KD - 1),
                        )
                    tt_u = attnp.tile(
                        [128, 512], BF16, tag="tt", bufs=KU + 1, name=f"tt{u}_{hf}"
                    )
                    for j in range(4):
                        b_ = 4 * hf + j
                        nc.scalar.activation(
                            tt_u[:, 128 * j : 128 * (j + 1)],
                            pa[:, 128 * j : 128 * (j + 1)],
                            AF.Tanh,
                            bias=qt[:, BL * u + b_ : BL * u + b_ + 1],
                        )
                    tt.append(tt_u)
                srow = prow.tile([1, 512], F32, tag="row", name=f"srow{hf}")
                for u in range(KU):
                    nc.tensor.matmul(
                        srow[:],
                        vt[:, u : u + 1],
                        tt[u][:],
                        start=(u == 0),
                        stop=(u == KU - 1),
                    )
                erow = []
                for j in range(4):
                    b_ = 4 * hf + j
                    er = smallp.tile([1, S], F32, tag="erow", bufs=4, name=f"erow{b_}")
                    nc.scalar.activation(
                        er[:],
                        srow[0:1, 128 * j : 128 * (j + 1)],
                        AF.Exp,
                        accum_out=esums[0:1, b_ : b_ + 1],
                    )
                    erow.append(er)
                nc.vector.reciprocal(
                    rsum[0:1, 4 * hf : 4 * (hf + 1)], esums[0:1, 4 * hf : 4 * (hf + 1)]
                )
                for j in range(4):
                    b_ = 4 * hf + j
                    wrow = smallp.tile([1, S], BF16, tag="wrow", bufs=3, name="wrow")
                    nc.vector.tensor_scalar_mul(
                        wrow[:], erow[j][:], rsum[0:1, b_ : b_ + 1]
                    )
                    pw = ptr.tile([128, 128], F32, tag="tr", bufs=2, name="pw")
                    nc.tensor.matmul(
                        pw[:, 0:1], wrow[:], ones_b[0:1, 0:1], start=True, stop=True
                    )
                    nc.vector.tensor_copy(wc8[:, b_ : b_ + 1], pw[:, 0:1])
                    csb = smallp.tile([1, DENC], F32, tag="csb", bufs=1, name="csb")
                    for n in range(2):
                        crow = prow.tile([1, 512], F32, tag="row", name="crow")
                        nc.tensor.matmul(
                            crow[:],
                            wc8[:, b_ : b_ + 1],
                            eall[:, DENC * b_ + 512 * n : DENC * b_ + 512 * (n + 1)],
                            start=True,
                            stop=True,
                        )
                        nc.vector.tensor_copy(
                            csb[0:1, 512 * n : 512 * (n + 1)], crow[:]
                        )
                    nc.scalar.dma_start(out=ctx_ag_in[b_ : b_ + 1, :], in_=csb[:])

            # ---------------- AllGather context ----------------
            nc.gpsimd.collective_compute(
                "AllGather",
                ALU.bypass,
                replica_groups=rg,
                ins=[ctx_ag_in[:].opt()],
                outs=[ctx_ag_out[:].opt()],
            )
            ctx_sb = smallp.tile([B, DENC], F32, tag="hbuf", bufs=1, name="ctx_sb")
            nc.sync.dma_start(out=ctx_sb[:], in_=ctx_ag_out[:])
            for d in range(KD):
                x = smallp.tile([128, B], F32R, name=f"xtc{d}")
                transpose_to(x[:], ctx_sb[:, 128 * d : 128 * (d + 1)], B, 128)
                xtk.append(x)

            # keep PE warm into the LSTM (keyed on first ctx chunk)
            warml = pacc.tile([B, 512], F32, tag="acc", name="warml")
            for wk in range(8):
                nc.tensor.matmul(
                    warml[:],
                    xtk[KE][:, 0:B],
                    w2all[:, 512 * (wk % KD) : 512 * (wk % KD + 1)],
                    start=(wk == 0),
                    stop=(wk == 7),
                )
            warml_sb = smallp.tile([1, 1], F32)
            nc.vector.tensor_copy(warml_sb[:], warml[0:1, 0:1])

            # ---------------- phase L: LSTM (natural layout, z=[64,512]) ----
            zp = pacc.tile([B, 4 * HC], F32, tag="acc", name="zp")
            for k in range(KE):
                nc.tensor.matmul(
                    zp[:],
                    xtk[k][:],
                    wxall[:, 4 * HC * k : 4 * HC * (k + 1)],
                    start=(k == 0),
                    stop=False,
                )
            for k in range(KH):
                nc.tensor.matmul(
                    zp[:],
                    htk[k],
                    whall[:, 4 * HC * k : 4 * HC * (k + 1)],
                    start=False,
                    stop=False,
                )
            nc.tensor.matmul(
                zp[:], ones_r[0:1, 0:B], b_sb[0:1, :], start=False, stop=False
            )
            for k in range(KE, KIN):
                nc.tensor.matmul(
                    zp[:],
                    xtk[k][:],
                    wxall[:, 4 * HC * k : 4 * HC * (k + 1)],
                    start=False,
                    stop=(k == KIN - 1),
                )
            gi = smallp.tile([B, HC], F32)
            gf = smallp.tile([B, HC], F32)
            gg = smallp.tile([B, HC], F32)
            go = smallp.tile([B, HC], F32)
            nc.scalar.activation(gi[:], zp[:, 0 * HC : 1 * HC], AF.Sigmoid)
            nc.scalar.activation(gf[:], zp[:, 1 * HC : 2 * HC], AF.Sigmoid)
            nc.scalar.activation(gg[:], zp[:, 2 * HC : 3 * HC], AF.Tanh)
            nc.scalar.activation(go[:], zp[:, 3 * HC : 4 * HC], AF.Sigmoid)
            t1 = smallp.tile([B, HC], F32)
            nc.vector.tensor_mul(t1[:], gf[:], c_sb[:])
            t2 = smallp.tile([B, HC], F32)
            nc.vector.tensor_mul(t2[:], gi[:], gg[:])
            cn = smallp.tile([B, HC], F32)
            nc.vector.tensor_add(cn[:], t1[:], t2[:])
            nc.scalar.dma_start(out=c_out[:, :], in_=cn[:])
            th = smallp.tile([B, HC], F32)
            nc.scalar.activation(th[:], cn[:], AF.Tanh)
            hn_nat = smallp.tile([B, HC], F32)
            nc.vector.tensor_mul(hn_nat[:], go[:], th[:])
            nc.scalar.dma_start(out=h_out[:, :], in_=hn_nat[:])
            hn_r = smallp.tile([128, B], F32R)
            transpose_to(hn_r[:], hn_nat[:, :], B, 128)
            nc.scalar.dma_start(out=hn_ag_in[:, :], in_=hn_r[:HC, :])

            # ---------------- AllGather h_new^T ----------------
            nc.gpsimd.collective_compute(
                "AllGather",
                ALU.bypass,
                replica_groups=rg,
                ins=[hn_ag_in[:].opt()],
                outs=[hn_ag_out[:].opt()],
            )
            # keep the PE HAM-warm through the AllGather gap
            warmp = pacc.tile([B, 512], F32, tag="acc", name="warmp")
            for wk in range(14):
                nc.tensor.matmul(
                    warmp[:],
                    hn_r[:, 0:B],
                    w2all[:, 512 * (wk % KD) : 512 * (wk % KD + 1)],
                    start=(wk == 0),
                    stop=(wk == 13),
                )
            warm_sb = smallp.tile([1, 1], F32)
            nc.vector.tensor_copy(warm_sb[:], warmp[0:1, 0:1])
            hnall_r = smallp.tile([128, KH * B], F32R)
            nc.sync.dma_start(
                out=hnall_r[:].rearrange("p (k b) -> p k b", k=KH),
                in_=hn_ag_out[:, :].rearrange("(k p) b -> p k b", p=128),
            )
            hnall = smallp.tile([128, KH * B], BF16)
            nc.vector.tensor_copy(hnall[:], hnall_r[:])

            # ---------------- phase F: fc + vocab softmax ----------------
            bfc_sb = epool.tile([1, VC], BF16, tag="ebig", bufs=1, name="bfc_sb")
            nc.scalar.dma_start(out=bfc_sb[:], in_=bfc[:])
            sums = []
            for half in range(2):
                sums.append(smallp.tile([B, 4], F32, name=f"sums{half}"))
            expall = attnp.tile([B, VC], F32, tag="big", bufs=1, name="expall")
            for n in range(NNC):
                wp = wfcp.tile([128, KH * NCH], BF16, tag="wfc", name=f"wp{n}")
                nc.sync.dma_start(
                    out=wp[:].rearrange("p (k j) -> p k j", k=KH),
                    in_=wfc[:, NCH * n : NCH * (n + 1)].rearrange(
                        "(k p) j -> p k j", p=128
                    ),
                )
                pf = pap.tile([B, NCH], F32, tag="pa", name=f"pf{n}")
                for k in range(KH):
                    nc.tensor.matmul(
                        pf[:],
                        hnall[:, B * k : B * (k + 1)],
                        wp[:, NCH * k : NCH * (k + 1)],
                        start=(k == 0),
                        stop=False,
                    )
                nc.tensor.matmul(
                    pf[:],
                    ones_b[0:1, 0:B],
                    bfc_sb[0:1, NCH * n : NCH * (n + 1)],
                    start=False,
                    stop=True,
                )
                nc.scalar.activation(
                    expall[:, NCH * n : NCH * (n + 1)],
                    pf[:],
                    AF.Exp,
                    accum_out=sums[n // 4][:, n % 4 : n % 4 + 1],
                )
            tot = smallp.tile([B, 2], F32)
            for half in range(2):
                nc.vector.tensor_reduce(
                    tot[:, half : half + 1], sums[half][:], AX.X, ALU.add
                )
            tsum = smallp.tile([B, 1], F32)
            nc.vector.tensor_add(tsum[:], tot[:, 0:1], tot[:, 1:2])
            nc.scalar.dma_start(out=ar_half[0][:, :], in_=tsum[:])
            nc.gpsimd.collective_compute(
                "AllReduce",
                ALU.add,
                replica_groups=rg,
                ins=[ar_half[0][:].opt()],
                outs=[ar_half_out[0][:].opt()],
            )
            ssum = smallp.tile([B, 1], F32)
            nc.scalar.dma_start(out=ssum[:], in_=ar_half_out[0][:, :])
            rv = smallp.tile([B, 1], F32)
            nc.vector.reciprocal(rv[:], ssum[:])
            nc.vector.tensor_scalar_mul(expall[:], expall[:], rv[:, 0:1])
            nc.sync.dma_start(out=probs_out[:, :], in_=expall[:])
    return nc


_NC_CACHE = None


def get_nc():
    global _NC_CACHE
    if _NC_CACHE is None:
        nc = bacc.Bacc(
            "TRN2", target_bir_lowering=False, debug=False, num_devices=N_CORES
        )
        build_kernel(nc)
        nc.compile()
        _NC_CACHE = nc
    return _NC_CACHE


def make_in_maps(token, enc_out, h, c, emb_table, W1, W2, V, Wx, Wh, b, Wfc, bfc):
    token = np.ascontiguousarray(np.asarray(token, dtype=np.int32))
    enc_out = np.asarray(enc_out, dtype=np.float32)
    h = np.asarray(h, dtype=np.float32)
    c = np.asarray(c, dtype=np.float32)
    emb_table = np.ascontiguousarray(np.asarray(emb_table, dtype=np.float32))
    W1 = np.ascontiguousarray(np.asarray(W1, dtype=np.float32))
    W2 = np.ascontiguousarray(np.asarray(W2, dtype=np.float32))
    V = np.ascontiguousarray(np.asarray(V, dtype=np.float32))
    Wx = np.asarray(Wx, dtype=np.float32)
    Wh = np.asarray(Wh, dtype=np.float32)
    b = np.asarray(b, dtype=np.float32)
    Wfc = np.asarray(Wfc, dtype=np.float32)
    bfc = np.asarray(bfc, dtype=np.float32)

    in_maps = []
    for k in range(N_CORES):
        hc = slice(HC * k, HC * (k + 1))
        gate_cols = np.concatenate(
            [np.arange(HC * k, HC * (k + 1)) + j * H for j in range(4)]
        )
        in_maps.append(
            {
                "enc": np.ascontiguousarray(enc_out[BL * k : BL * (k + 1)]).astype(
                    ml_dtypes.bfloat16
                ),
                "enc_t": np.ascontiguousarray(
                    enc_out[BL * k : BL * (k + 1)]
                    .transpose(2, 0, 1)
                    .reshape(DENC, BL * S)
                ).astype(ml_dtypes.bfloat16),
                "hT": np.ascontiguousarray(h.T),
                "h_locT": np.ascontiguousarray(h[BL * k : BL * (k + 1)].T),
                "token": token,
                "emb_table": emb_table,
                "c_sl": np.ascontiguousarray(c[:, hc]),
                "W1": W1.astype(ml_dtypes.bfloat16),
                "W2": W2,
                "V": V.astype(ml_dtypes.bfloat16),
                "Wx_sl": np.ascontiguousarray(Wx[:, gate_cols]),
                "Wh_sl": np.ascontiguousarray(Wh[:, gate_cols]),
                "b_sl": np.ascontiguousarray(b[gate_cols]).reshape(1, 4 * HC),
                "Wfc_sl": np.ascontiguousarray(
                    Wfc[:, VC * k : VC * (k + 1)]
                ).astype(ml_dtypes.bfloat16),
                "bfc_sl": np.ascontiguousarray(bfc[VC * k : VC * (k + 1)])
                .reshape(1, VC)
                .astype(ml_dtypes.bfloat16),
            }
        )
    return in_maps


def assemble(results):
    probs = np.concatenate([results[k]["probs"] for k in range(N_CORES)], axis=1)
    h_new = np.concatenate([results[k]["h_sl"] for k in range(N_CORES)], axis=1)
    c_new = np.concatenate([results[k]["c_sl_out"] for k in range(N_CORES)], axis=1)
    return probs, h_new, c_new


def kernel(token, enc_out, h, c, emb_table, W1, W2, V, Wx, Wh, b, Wfc, bfc):
    nc = get_nc()
    in_maps = make_in_maps(
        token, enc_out, h, c, emb_table, W1, W2, V, Wx, Wh, b, Wfc, bfc
    )
    res = run_bass_kernel_spmd(nc, in_maps, list(range(N_CORES))).results
    return assemble(res)


# revision 30
# speedup vs baseline: 1.0081x; 1.0081x over previous
"""Trainium2 Bass kernel for nn_DecoderCell: embedding gather -> Bahdanau
attention -> single-step LSTM -> vocab softmax, distributed over 8 NeuronCores.

Sharding (hardcoded, per spec sharding_hint):
  - attention: batch-sharded (8 batches/core, enc_out slice 4MB/core)
  - LSTM: hidden-dim sharded (128 h-dims/core -> 4x128 gate columns/core)
  - fc/softmax: vocab-sharded (4000 vocab cols/core, Wfc slice 16MB/core)
  - collectives: AllGather(context rows), AllGather(h_new^T), AllReduce(softmax sum)

kernel(**inputs) takes FULL unsharded inputs, returns (probs, h_new, c_new).
"""
import numpy as np
import ml_dtypes

import concourse.bacc as bacc
import concourse.bass as bass
import concourse.tile as tile
from concourse import mybir
from concourse.bass_utils import run_bass_kernel_spmd
from concourse.masks import make_identity

F32 = mybir.dt.float32
F32R = mybir.dt.float32r
BF16 = mybir.dt.bfloat16
I32 = mybir.dt.int32
AF = mybir.ActivationFunctionType
ALU = mybir.AluOpType
AX = mybir.AxisListType

N_CORES = 8
B, S, VOCAB, EMB, UNITS = 64, 128, 32000, 256, 512
H = 2 * UNITS            # 1024
DENC = 2 * UNITS         # 1024
IN = EMB + DENC          # 1280
BL = B // N_CORES        # 8 local batches
HC = H // N_CORES        # 128 h-dims per core
VC = VOCAB // N_CORES    # 4000 vocab cols per core
NCH = 500                # fc column chunk (<=512 psum bank)
NNC = VC // NCH          # 8 fc chunks
KD = DENC // 128         # 8
KH = H // 128            # 8
KU = UNITS // 128        # 4
KE = EMB // 128          # 2
KIN = IN // 128          # 10
GRP = 2                  # attention batch groups
GB = BL // GRP           # 4 batches per group


def build_kernel(nc):
    enc = nc.dram_tensor("enc", [BL, S, DENC], BF16, kind="ExternalInput")
    enc_t = nc.dram_tensor("enc_t", [DENC, BL * S], BF16, kind="ExternalInput")
    h_t = nc.dram_tensor("hT", [H, B], F32R, kind="ExternalInput")
    hl_t = nc.dram_tensor("h_locT", [H, BL], F32R, kind="ExternalInput")
    token = nc.dram_tensor("token", [B, 1], I32, kind="ExternalInput")
    emb_table = nc.dram_tensor("emb_table", [VOCAB, EMB], F32, kind="ExternalInput")
    c_sl = nc.dram_tensor("c_sl", [B, HC], F32, kind="ExternalInput")
    w1 = nc.dram_tensor("W1", [DENC, UNITS], BF16, kind="ExternalInput")
    w2 = nc.dram_tensor("W2", [DENC, UNITS], F32R, kind="ExternalInput")
    v_in = nc.dram_tensor("V", [UNITS, 1], BF16, kind="ExternalInput")
    wx = nc.dram_tensor("Wx_sl", [IN, 4 * HC], F32R, kind="ExternalInput")
    wh = nc.dram_tensor("Wh_sl", [H, 4 * HC], F32R, kind="ExternalInput")
    b_sl = nc.dram_tensor("b_sl", [1, 4 * HC], F32R, kind="ExternalInput")
    wfc = nc.dram_tensor("Wfc_sl", [H, VC], BF16, kind="ExternalInput")
    bfc = nc.dram_tensor("bfc_sl", [1, VC], BF16, kind="ExternalInput")

    probs_out = nc.dram_tensor("probs", [B, VC], F32, kind="ExternalOutput")
    h_out = nc.dram_tensor("h_sl", [B, HC], F32, kind="ExternalOutput")
    c_out = nc.dram_tensor("c_sl_out", [B, HC], F32, kind="ExternalOutput")

    rg = [list(range(N_CORES))]

    with tile.TileContext(nc) as tc:
        with (
            tc.tile_pool(name="dram", bufs=1, space="DRAM") as dram,
            tc.tile_pool(name="const", bufs=1) as constp,
            tc.tile_pool(name="wt", bufs=1) as wtp,
            tc.tile_pool(name="small", bufs=1) as smallp,
            tc.tile_pool(name="attn", bufs=1) as attnp,
            tc.tile_pool(name="epool", bufs=5) as epool,
            tc.tile_pool(name="wfcp", bufs=4) as wfcp,
            tc.tile_pool(name="expp", bufs=1) as expp,
            tc.tile_pool(name="ptr", bufs=2, space="PSUM") as ptr,
            tc.tile_pool(name="pacc", bufs=1, space="PSUM") as pacc,
            tc.tile_pool(name="pa", bufs=3, space="PSUM") as pap,
            tc.tile_pool(name="prow", bufs=2, space="PSUM") as prow,
        ):
            # DRAM bounce buffers for collectives
            dum_in = dram.tile([1, 8], F32)
            dum_out = dram.tile([N_CORES, 8], F32)
            dum2_in = dram.tile([1, 8], F32)
            dum2_out = dram.tile([1, 8], F32)
            ctx_ag_in = dram.tile([BL, DENC], F32)
            ctx_ag_out = dram.tile([B, DENC], F32)
            hn_ag_in = dram.tile([HC, B], F32R)
            hn_ag_out = dram.tile([H, B], F32R)
            ar_half = [dram.tile([B, 1], F32, name=f"arh{i}") for i in range(2)]
            ar_half_out = [dram.tile([B, 1], F32, name=f"arho{i}") for i in range(2)]

            # warm up the collective path (absorbs the first-collective global
            # barrier + ncfw init while input DMAs stream)
            dum_sb = smallp.tile([1, 8], F32)
            nc.vector.memset(dum_sb[:], 0.0)
            nc.scalar.dma_start(out=dum_in[:], in_=dum_sb[:])
            nc.gpsimd.collective_compute(
                "AllGather", ALU.bypass, replica_groups=rg,
                ins=[dum_in[:].opt()], outs=[dum_out[:].opt()],
            )
            nc.scalar.dma_start(out=dum2_in[:], in_=dum_sb[:])
            nc.gpsimd.collective_compute(
                "AllReduce", ALU.add, replica_groups=rg,
                ins=[dum2_in[:].opt()], outs=[dum2_out[:].opt()],
            )

            # constants
            ident = constp.tile([128, 128], F32)
            make_identity(nc, ident[:])
            ones_f = constp.tile([1, B], F32)
            nc.vector.memset(ones_f[:], 1.0)
            ones_r = constp.tile([1, B], F32R)
            nc.vector.tensor_copy(ones_r[:], ones_f[:])
            ones_b = constp.tile([1, B], BF16)
            nc.vector.tensor_copy(ones_b[:], ones_f[:])

            # ---------------- phase 0: loads ----------------
            tok = smallp.tile([B, 1], I32)
            nc.scalar.dma_start(out=tok[:], in_=token[:])
            emb_sb = smallp.tile([B, EMB], F32)
            nc.gpsimd.indirect_dma_start(
                out=emb_sb[:],
                out_offset=None,
                in_=emb_table[:],
                in_offset=bass.IndirectOffsetOnAxis(ap=tok[:, :1], axis=0),
            )
            c_sb = smallp.tile([B, HC], F32)
            nc.scalar.dma_start(out=c_sb[:], in_=c_sl[:])

            w1all = wtp.tile([128, KD * UNITS], BF16)
            nc.sync.dma_start(
                out=w1all[:].rearrange("p (k u) -> p k u", k=KD),
                in_=w1[:, :].rearrange("(k p) u -> p k u", p=128),
            )
            w2all = wtp.tile([128, KD * UNITS], F32R)
            nc.scalar.dma_start(
                out=w2all[:].rearrange("p (k u) -> p k u", k=KD),
                in_=w2[:, :].rearrange("(k p) u -> p k u", p=128),
            )
            vt = wtp.tile([128, KU], BF16)
            nc.scalar.dma_start(
                out=vt[:].rearrange("p (k o) -> p k o", k=KU),
                in_=v_in[:, :].rearrange("(k p) o -> p k o", p=128),
            )
            wxall = wtp.tile([128, KIN * 4 * HC], F32R)
            nc.sync.dma_start(
                out=wxall[:].rearrange("p (k u) -> p k u", k=KIN),
                in_=wx[:, :].rearrange("(k p) u -> p k u", p=128),
            )
            whall = wtp.tile([128, KH * 4 * HC], F32R)
            nc.scalar.dma_start(
                out=whall[:].rearrange("p (k u) -> p k u", k=KH),
                in_=wh[:, :].rearrange("(k p) u -> p k u", p=128),
            )
            b_sb = smallp.tile([1, 4 * HC], F32R)
            nc.scalar.dma_start(out=b_sb[:], in_=b_sl[:])

            # ---------------- phase T: input transposes ----------------
            def transpose_to(dst_ap, src_ap, p_in, p_out):
                tr = ptr.tile([128, 128], F32, tag="tr", bufs=2, name="tr")
                nc.tensor.transpose(tr[:p_out, :p_in], src_ap, ident[:p_in, :p_in])
                nc.vector.tensor_copy(dst_ap, tr[:p_out, :p_in])

            htall = smallp.tile([128, KH * B], F32R)
            nc.sync.dma_start(
                out=htall[:].rearrange("p (k b) -> p k b", k=KH),
                in_=h_t[:, :].rearrange("(k p) b -> p k b", p=128),
            )
            htk = [htall[:, B * k : B * (k + 1)] for k in range(KH)]
            hlall = smallp.tile([128, KH * BL], F32R)
            nc.scalar.dma_start(
                out=hlall[:].rearrange("p (k b) -> p k b", k=KH),
                in_=hl_t[:, :].rearrange("(k p) b -> p k b", p=128),
            )
            hlk = [hlall[:, BL * k : BL * (k + 1)] for k in range(KH)]
            xtk = []  # x^T chunks [128, 64] f32r: emb (2) then ctx (8)
            for k in range(KE):
                d = smallp.tile([128, B], F32R, name=f"xtk{k}")
                transpose_to(d[:], emb_sb[:, 128 * k : 128 * (k + 1)], B, 128)
                xtk.append(d)

            # ---------------- phase Q: q = h_loc @ W2 (natural layout) -------
            qp = pacc.tile([BL, UNITS], F32, tag="acc", name="qp")
            for k in range(KH):
                nc.tensor.matmul(
                    qp[:],
                    hlk[k],
                    w2all[:, UNITS * k : UNITS * (k + 1)],
                    start=(k == 0),
                    stop=(k == KH - 1),
                )
            q_sb = smallp.tile([BL, UNITS], F32)
            nc.vector.tensor_copy(q_sb[:], qp[:])
            qt = smallp.tile([128, KU * BL], F32)  # q^T: [128(u), 8] per u-chunk
            for u in range(KU):
                transpose_to(
                    qt[:, BL * u : BL * (u + 1)],
                    q_sb[:, 128 * u : 128 * (u + 1)],
                    BL,
                    128,
                )

            # ---------------- phase A: attention (bf16, DMA-transposed E) ----
            esums = smallp.tile([1, BL], F32)
            rsum = smallp.tile([1, BL], F32)
            wc8 = smallp.tile([128, BL], BF16)

            etall = attnp.tile([128, KD * BL * S], BF16, tag="big", bufs=1)
            for hf in range(2):
                nc.sync.dma_start(
                    out=etall[:].rearrange("p (k t c) -> p k t c", k=KD, t=2)[
                        :, :, hf, :
                    ],
                    in_=enc_t[:, 512 * hf : 512 * (hf + 1)].rearrange(
                        "(k p) c -> p k c", p=128
                    ),
                )
            etg = [
                [etall[:, 1024 * d + 512 * hf : 1024 * d + 512 * (hf + 1)] for hf in range(2)]
                for d in range(KD)
            ]
            eall = epool.tile([S, BL * DENC], BF16, tag="ebig", bufs=1)
            nc.sync.dma_start(
                out=eall[:].rearrange("s (b d) -> s b d", b=BL),
                in_=enc[:, :, :].rearrange("b s d -> s b d"),
            )
            e_tiles = [eall[:, DENC * b_ : DENC * (b_ + 1)] for b_ in range(BL)]

            for hf in range(2):
                tt = []
                for u in range(KU):
                    pa = pap.tile([128, 512], F32, tag="pa", name=f"pa{u}_{hf}")
                    for k in range(KD):
                        nc.tensor.matmul(
                            pa[:],
                            w1all[:, UNITS * k + 128 * u : UNITS * k + 128 * (u + 1)],
                            etg[k][hf][:],
                            start=(k == 0),
                            stop=(k == KD - 1),
                        )
                    tt_u = attnp.tile(
                        [128, 512], BF16, tag="tt", bufs=KU + 1, name=f"tt{u}_{hf}"
                    )
                    for j in range(4):
                        b_ = 4 * hf + j
                        nc.scalar.activation(
                            tt_u[:, 128 * j : 128 * (j + 1)],
                            pa[:, 128 * j : 128 * (j + 1)],
                            AF.Tanh,
                            bias=qt[:, BL * u + b_ : BL * u + b_ + 1],
                        )
                    tt.append(tt_u)
                srow = prow.tile([1, 512], F32, tag="row", name=f"srow{hf}")
                for u in range(KU):
                    nc.tensor.matmul(
                        srow[:],
                        vt[:, u : u + 1],
                        tt[u][:],
                        start=(u == 0),
                        stop=(u == KU - 1),
                    )
                erow = []
                for j in range(4):
                    b_ = 4 * hf + j
                    er = smallp.tile([1, S], F32, tag="erow", bufs=4, name=f"erow{b_}")
                    nc.scalar.activation(
                        er[:],
                        srow[0:1, 128 * j : 128 * (j + 1)],
                        AF.Exp,
                        accum_out=esums[0:1, b_ : b_ + 1],
                    )
                    erow.append(er)
                nc.vector.reciprocal(
                    rsum[0:1, 4 * hf : 4 * (hf + 1)], esums[0:1, 4 * hf : 4 * (hf + 1)]
                )
                for j in range(4):
                    b_ = 4 * hf + j
                    wrow = smallp.tile([1, S], BF16, tag="wrow", bufs=3, name="wrow")
                    nc.vector.tensor_scalar_mul(
                        wrow[:], erow[j][:], rsum[0:1, b_ : b_ + 1]
                    )
                    pw = ptr.tile([128, 128], F32, tag="tr", bufs=2, name="pw")
                    nc.tensor.matmul(
                        pw[:, 0:1], wrow[:], ones_b[0:1, 0:1], start=True, stop=True
                    )
                    nc.vector.tensor_copy(wc8[:, b_ : b_ + 1], pw[:, 0:1])
                    csb = smallp.tile([1, DENC], F32, tag="csb", bufs=2, name="csb")
                    for n in range(2):
                        crow = prow.tile([1, 512], F32, tag="row", name="crow")
                        nc.tensor.matmul(
                            crow[:],
                            wc8[:, b_ : b_ + 1],
                            eall[:, DENC * b_ + 512 * n : DENC * b_ + 512 * (n + 1)],
                            start=True,
                            stop=True,
                        )
                        nc.vector.tensor_copy(
                            csb[0:1, 512 * n : 512 * (n + 1)], crow[:]
                        )
                    nc.scalar.dma_start(out=ctx_ag_in[b_ : b_ + 1, :], in_=csb[:])

            # ---------------- AllGather context ----------------
            nc.gpsimd.collective_compute(
                "AllGather",
                ALU.bypass,
                replica_groups=rg,
                ins=[ctx_ag_in[:].opt()],
                outs=[ctx_ag_out[:].opt()],
            )
            ctx_sb = smallp.tile([B, DENC], F32, tag="hbuf", bufs=1, name="ctx_sb")
            nc.sync.dma_start(out=ctx_sb[:], in_=ctx_ag_out[:])
            for d in range(KD):
                x = smallp.tile([128, B], F32R, name=f"xtc{d}")
                transpose_to(x[:], ctx_sb[:, 128 * d : 128 * (d + 1)], B, 128)
                xtk.append(x)

            # keep PE warm into the LSTM (keyed on first ctx chunk)
            warml = pacc.tile([B, 512], F32, tag="acc", name="warml")
            for wk in range(8):
                nc.tensor.matmul(
                    warml[:],
                    xtk[KE][:, 0:B],
                    w2all[:, 512 * (wk % KD) : 512 * (wk % KD + 1)],
                    start=(wk == 0),
                    stop=(wk == 7),
                )
            warml_sb = smallp.tile([1, 1], F32)
            nc.vector.tensor_copy(warml_sb[:], warml[0:1, 0:1])

            # ---------------- phase L: LSTM (natural layout, z=[64,512]) ----
            zp = pacc.tile([B, 4 * HC], F32, tag="acc", name="zp")
            for k in range(KE):
                nc.tensor.matmul(
                    zp[:],
                    xtk[k][:],
                    wxall[:, 4 * HC * k : 4 * HC * (k + 1)],
                    start=(k == 0),
                    stop=False,
                )
            for k in range(KH):
                nc.tensor.matmul(
                    zp[:],
                    htk[k],
                    whall[:, 4 * HC * k : 4 * HC * (k + 1)],
                    start=False,
                    stop=False,
                )
            nc.tensor.matmul(
                zp[:], ones_r[0:1, 0:B], b_sb[0:1, :], start=False, stop=False
            )
            for k in range(KE, KIN):
                nc.tensor.matmul(
                    zp[:],
                    xtk[k][:],
                    wxall[:, 4 * HC * k : 4 * HC * (k + 1)],
                    start=False,
                    stop=(k == KIN - 1),
                )
            gi = smallp.tile([B, HC], F32)
            gf = smallp.tile([B, HC], F32)
            gg = smallp.tile([B, HC], F32)
            go = smallp.tile([B, HC], F32)
            nc.scalar.activation(gi[:], zp[:, 0 * HC : 1 * HC], AF.Sigmoid)
            nc.scalar.activation(gf[:], zp[:, 1 * HC : 2 * HC], AF.Sigmoid)
            nc.scalar.activation(gg[:], zp[:, 2 * HC : 3 * HC], AF.Tanh)
            nc.scalar.activation(go[:], zp[:, 3 * HC : 4 * HC], AF.Sigmoid)
            t1 = smallp.tile([B, HC], F32)
            nc.vector.tensor_mul(t1[:], gf[:], c_sb[:])
            t2 = smallp.tile([B, HC], F32)
            nc.vector.tensor_mul(t2[:], gi[:], gg[:])
            cn = smallp.tile([B, HC], F32)
            nc.vector.tensor_add(cn[:], t1[:], t2[:])
            nc.scalar.dma_start(out=c_out[:, :], in_=cn[:])
            th = smallp.tile([B, HC], F32)
            nc.scalar.activation(th[:], cn[:], AF.Tanh)
            hn_nat = smallp.tile([B, HC], F32)
            nc.vector.tensor_mul(hn_nat[:], go[:], th[:])
            nc.scalar.dma_start(out=h_out[:, :], in_=hn_nat[:])
            hn_r = smallp.tile([128, B], F32R)
            transpose_to(hn_r[:], hn_nat[:, :], B, 128)
            nc.scalar.dma_start(out=hn_ag_in[:, :], in_=hn_r[:HC, :])

            # ---------------- AllGather h_new^T ----------------
            nc.gpsimd.collective_compute(
                "AllGather",
                ALU.bypass,
                replica_groups=rg,
                ins=[hn_ag_in[:].opt()],
                outs=[hn_ag_out[:].opt()],
            )
            # keep the PE HAM-warm through the AllGather gap
            warmp = pacc.tile([B, 512], F32, tag="acc", name="warmp")
            for wk in range(14):
                nc.tensor.matmul(
                    warmp[:],
                    hn_r[:, 0:B],
                    w2all[:, 512 * (wk % KD) : 512 * (wk % KD + 1)],
                    start=(wk == 0),
                    stop=(wk == 13),
                )
            warm_sb = smallp.tile([1, 1], F32)
            nc.vector.tensor_copy(warm_sb[:], warmp[0:1, 0:1])
            hnall_r = smallp.tile([128, KH * B], F32R)
            nc.sync.dma_start(
                out=hnall_r[:].rearrange("p (k b) -> p k b", k=KH),
                in_=hn_ag_out[:, :].rearrange("(k p) b -> p k b", p=128),
            )
            hnall = smallp.tile([128, KH * B], BF16)
            nc.vector.tensor_copy(hnall[:], hnall_r[:])

            # ---------------- phase F: fc + vocab softmax ----------------
            bfc_sb = epool.tile([1, VC], BF16, tag="ebig", bufs=1, name="bfc_sb")
            nc.scalar.dma_start(out=bfc_sb[:], in_=bfc[:])
            sums = []
            for half in range(2):
                sums.append(smallp.tile([B, 4], F32, name=f"sums{half}"))
            expall = attnp.tile([B, VC], F32, tag="big", bufs=1, name="expall")
            for n in range(NNC):
                wp = wfcp.tile([128, KH * NCH], BF16, tag="wfc", name=f"wp{n}")
                nc.sync.dma_start(
                    out=wp[:].rearrange("p (k j) -> p k j", k=KH),
                    in_=wfc[:, NCH * n : NCH * (n + 1)].rearrange(
                        "(k p) j -> p k j", p=128
                    ),
                )
                pf = pap.tile([B, NCH], F32, tag="pa", name=f"pf{n}")
                for k in range(KH):
                    nc.tensor.matmul(
                        pf[:],
                        hnall[:, B * k : B * (k + 1)],
                        wp[:, NCH * k : NCH * (k + 1)],
                        start=(k == 0),
                        stop=False,
                    )
                nc.tensor.matmul(
                    pf[:],
                    ones_b[0:1, 0:B],
                    bfc_sb[0:1, NCH * n : NCH * (n + 1)],
                    start=False,
                    stop=True,
                )
                nc.scalar.activation(
                    expall[:, NCH * n : NCH * (n + 1)],
                    pf[:],
                    AF.Exp,
                    accum_out=sums[n // 4][:, n % 4 : n % 4 + 1],
                )
            tot = smallp.tile([B, 2], F32)
            for half in range(2):
                nc.vector.tensor_reduce(
                    tot[:, half : half + 1], sums[half][:], AX.X, ALU.add
                )
            tsum = smallp.tile([B, 1], F32)
            nc.vector.tensor_add(tsum[:], tot[:, 0:1], tot[:, 1:2])
            nc.scalar.dma_start(out=ar_half[0][:, :], in_=tsum[:])
            nc.gpsimd.collective_compute(
                "AllReduce",
                ALU.add,
                replica_groups=rg,
                ins=[ar_half[0][:].opt()],
                outs=[ar_half_out[0][:].opt()],
            )
            ssum = smallp.tile([B, 1], F32)
            nc.scalar.dma_start(out=ssum[:], in_=ar_half_out[0][:, :])
            rv = smallp.tile([B, 1], F32)
            nc.vector.reciprocal(rv[:], ssum[:])
            nc.vector.tensor_scalar_mul(expall[:], expall[:], rv[:, 0:1])
            nc.sync.dma_start(out=probs_out[:, :], in_=expall[:])
    return nc


_NC_CACHE = None


def get_nc():
    global _NC_CACHE
    if _NC_CACHE is None:
        nc = bacc.Bacc(
            "TRN2", target_bir_lowering=False, debug=False, num_devices=N_CORES
        )
        build_kernel(nc)
        nc.compile()
        _NC_CACHE = nc
    return _NC_CACHE


def make_in_maps(token, enc_out, h, c, emb_table, W1, W2, V, Wx, Wh, b, Wfc, bfc):
    token = np.ascontiguousarray(np.asarray(token, dtype=np.int32))
    enc_out = np.asarray(enc_out, dtype=np.float32)
    h = np.asarray(h, dtype=np.float32)
    c = np.asarray(c, dtype=np.float32)
    emb_table = np.ascontiguousarray(np.asarray(emb_table, dtype=np.float32))
    W1 = np.ascontiguousarray(np.asarray(W1, dtype=np.float32))
    W2 = np.ascontiguousarray(np.asarray(W2, dtype=np.float32))
    V = np.ascontiguousarray(np.asarray(V, dtype=np.float32))
    Wx = np.asarray(Wx, dtype=np.float32)
    Wh = np.asarray(Wh, dtype=np.float32)
    b = np.asarray(b, dtype=np.float32)
    Wfc = np.asarray(Wfc, dtype=np.float32)
    bfc = np.asarray(bfc, dtype=np.float32)

    in_maps = []
    for k in range(N_CORES):
        hc = slice(HC * k, HC * (k + 1))
        gate_cols = np.concatenate(
            [np.arange(HC * k, HC * (k + 1)) + j * H for j in range(4)]
        )
        in_maps.append(
            {
                "enc": np.ascontiguousarray(enc_out[BL * k : BL * (k + 1)]).astype(
                    ml_dtypes.bfloat16
                ),
                "enc_t": np.ascontiguousarray(
                    enc_out[BL * k : BL * (k + 1)]
                    .transpose(2, 0, 1)
                    .reshape(DENC, BL * S)
                ).astype(ml_dtypes.bfloat16),
                "hT": np.ascontiguousarray(h.T),
                "h_locT": np.ascontiguousarray(h[BL * k : BL * (k + 1)].T),
                "token": token,
                "emb_table": emb_table,
                "c_sl": np.ascontiguousarray(c[:, hc]),
                "W1": W1.astype(ml_dtypes.bfloat16),
                "W2": W2,
                "V": V.astype(ml_dtypes.bfloat16),
                "Wx_sl": np.ascontiguousarray(Wx[:, gate_cols]),
                "Wh_sl": np.ascontiguousarray(Wh[:, gate_cols]),
                "b_sl": np.ascontiguousarray(b[gate_cols]).reshape(1, 4 * HC),
                "Wfc_sl": np.ascontiguousarray(
                    Wfc[:, VC * k : VC * (k + 1)]
                ).astype(ml_dtypes.bfloat16),
                "bfc_sl": np.ascontiguousarray(bfc[VC * k : VC * (k + 1)])
                .reshape(1, VC)
                .astype(ml_dtypes.bfloat16),
            }
        )
    return in_maps


def assemble(results):
    probs = np.concatenate([results[k]["probs"] for k in range(N_CORES)], axis=1)
    h_new = np.concatenate([results[k]["h_sl"] for k in range(N_CORES)], axis=1)
    c_new = np.concatenate([results[k]["c_sl_out"] for k in range(N_CORES)], axis=1)
    return probs, h_new, c_new


def kernel(token, enc_out, h, c, emb_table, W1, W2, V, Wx, Wh, b, Wfc, bfc):
    nc = get_nc()
    in_maps = make_in_maps(
        token, enc_out, h, c, emb_table, W1, W2, V, Wx, Wh, b, Wfc, bfc
    )
    res = run_bass_kernel_spmd(nc, in_maps, list(range(N_CORES))).results
    return assemble(res)


# revision 31
# speedup vs baseline: 1.1151x; 1.1062x over previous
"""Trainium2 Bass kernel for nn_DecoderCell: embedding gather -> Bahdanau
attention -> single-step LSTM -> vocab softmax, distributed over 8 NeuronCores.

Sharding (hardcoded, per spec sharding_hint):
  - attention: batch-sharded (8 batches/core, enc_out slice 4MB/core)
  - LSTM: hidden-dim sharded (128 h-dims/core -> 4x128 gate columns/core)
  - fc/softmax: vocab-sharded (4000 vocab cols/core, Wfc slice 16MB/core)
  - collectives: AllGather(context rows), AllGather(h_new^T), AllReduce(softmax sum)

kernel(**inputs) takes FULL unsharded inputs, returns (probs, h_new, c_new).
"""
import numpy as np
import ml_dtypes

import concourse.bacc as bacc
import concourse.bass as bass
import concourse.tile as tile
from concourse import mybir
from concourse.bass_utils import run_bass_kernel_spmd
from concourse.masks import make_identity

F32 = mybir.dt.float32
F32R = mybir.dt.float32r
BF16 = mybir.dt.bfloat16
I32 = mybir.dt.int32
AF = mybir.ActivationFunctionType
ALU = mybir.AluOpType
AX = mybir.AxisListType

N_CORES = 8
B, S, VOCAB, EMB, UNITS = 64, 128, 32000, 256, 512
H = 2 * UNITS            # 1024
DENC = 2 * UNITS         # 1024
IN = EMB + DENC          # 1280
BL = B // N_CORES        # 8 local batches
HC = H // N_CORES        # 128 h-dims per core
VC = VOCAB // N_CORES    # 4000 vocab cols per core
NCH = 500                # fc column chunk (<=512 psum bank)
NNC = VC // NCH          # 8 fc chunks
KD = DENC // 128         # 8
KH = H // 128            # 8
KU = UNITS // 128        # 4
KE = EMB // 128          # 2
KIN = IN // 128          # 10
GRP = 2                  # attention batch groups
GB = BL // GRP           # 4 batches per group


def build_kernel(nc):
    enc = nc.dram_tensor("enc", [BL, S, DENC], BF16, kind="ExternalInput")
    enc_t = nc.dram_tensor("enc_t", [DENC, BL * S], BF16, kind="ExternalInput")
    h_t = nc.dram_tensor("hT", [H, B], F32R, kind="ExternalInput")
    hl_t = nc.dram_tensor("h_locT", [H, BL], F32R, kind="ExternalInput")
    token = nc.dram_tensor("token", [B, 1], I32, kind="ExternalInput")
    emb_table = nc.dram_tensor("emb_table", [VOCAB, EMB], F32, kind="ExternalInput")
    c_sl = nc.dram_tensor("c_sl", [B, HC], F32, kind="ExternalInput")
    w1 = nc.dram_tensor("W1", [DENC, UNITS], BF16, kind="ExternalInput")
    w2 = nc.dram_tensor("W2", [DENC, UNITS], F32R, kind="ExternalInput")
    v_in = nc.dram_tensor("V", [UNITS, 1], BF16, kind="ExternalInput")
    wx = nc.dram_tensor("Wx_sl", [IN, 4 * HC], F32R, kind="ExternalInput")
    wh = nc.dram_tensor("Wh_sl", [H, 4 * HC], F32R, kind="ExternalInput")
    b_sl = nc.dram_tensor("b_sl", [1, 4 * HC], F32R, kind="ExternalInput")
    wfc = nc.dram_tensor("Wfc_sl", [H, VC], BF16, kind="ExternalInput")
    bfc = nc.dram_tensor("bfc_sl", [1, VC], BF16, kind="ExternalInput")

    probs_out = nc.dram_tensor("probs", [B, VC], F32, kind="ExternalOutput")
    h_out = nc.dram_tensor("h_sl", [B, HC], F32, kind="ExternalOutput")
    c_out = nc.dram_tensor("c_sl_out", [B, HC], F32, kind="ExternalOutput")

    rg = [list(range(N_CORES))]

    with tile.TileContext(nc) as tc:
        with (
            tc.tile_pool(name="dram", bufs=1, space="DRAM") as dram,
            tc.tile_pool(name="const", bufs=1) as constp,
            tc.tile_pool(name="wt", bufs=1) as wtp,
            tc.tile_pool(name="small", bufs=1) as smallp,
            tc.tile_pool(name="attn", bufs=1) as attnp,
            tc.tile_pool(name="epool", bufs=5) as epool,
            tc.tile_pool(name="wfcp", bufs=5) as wfcp,
            tc.tile_pool(name="expp", bufs=1) as expp,
            tc.tile_pool(name="ptr", bufs=2, space="PSUM") as ptr,
            tc.tile_pool(name="pacc", bufs=1, space="PSUM") as pacc,
            tc.tile_pool(name="pa", bufs=3, space="PSUM") as pap,
            tc.tile_pool(name="prow", bufs=2, space="PSUM") as prow,
        ):
            # DRAM bounce buffers for collectives
            dum_in = dram.tile([1, 8], F32)
            dum_out = dram.tile([N_CORES, 8], F32)
            dum2_in = dram.tile([1, 8], F32)
            dum2_out = dram.tile([1, 8], F32)
            ctx_ag_in = dram.tile([BL, DENC], F32)
            ctx_ag_out = dram.tile([B, DENC], F32)
            hn_ag_in = dram.tile([HC, B], F32R)
            hn_ag_out = dram.tile([H, B], F32R)
            ar_half = [dram.tile([B, 1], F32, name=f"arh{i}") for i in range(2)]
            ar_half_out = [dram.tile([B, 1], F32, name=f"arho{i}") for i in range(2)]

            # warm up the collective path (absorbs the first-collective global
            # barrier + ncfw init while input DMAs stream)
            dum_sb = smallp.tile([1, 8], F32)
            nc.vector.memset(dum_sb[:], 0.0)
            nc.scalar.dma_start(out=dum_in[:], in_=dum_sb[:])
            nc.gpsimd.collective_compute(
                "AllGather", ALU.bypass, replica_groups=rg,
                ins=[dum_in[:].opt()], outs=[dum_out[:].opt()],
            )

            # constants
            ident = constp.tile([128, 128], F32)
            make_identity(nc, ident[:])
            ones_f = constp.tile([1, B], F32)
            nc.vector.memset(ones_f[:], 1.0)
            ones_r = constp.tile([1, B], F32R)
            nc.vector.tensor_copy(ones_r[:], ones_f[:])
            ones_b = constp.tile([1, B], BF16)
            nc.vector.tensor_copy(ones_b[:], ones_f[:])

            # ---------------- phase 0: loads ----------------
            tok = smallp.tile([B, 1], I32)
            nc.scalar.dma_start(out=tok[:], in_=token[:])
            emb_sb = smallp.tile([B, EMB], F32)
            nc.gpsimd.indirect_dma_start(
                out=emb_sb[:],
                out_offset=None,
                in_=emb_table[:],
                in_offset=bass.IndirectOffsetOnAxis(ap=tok[:, :1], axis=0),
            )
            c_sb = smallp.tile([B, HC], F32)
            nc.scalar.dma_start(out=c_sb[:], in_=c_sl[:])

            w1all = wtp.tile([128, KD * UNITS], BF16)
            nc.sync.dma_start(
                out=w1all[:].rearrange("p (k u) -> p k u", k=KD),
                in_=w1[:, :].rearrange("(k p) u -> p k u", p=128),
            )
            w2all = wtp.tile([128, KD * UNITS], F32R)
            nc.scalar.dma_start(
                out=w2all[:].rearrange("p (k u) -> p k u", k=KD),
                in_=w2[:, :].rearrange("(k p) u -> p k u", p=128),
            )
            vt = wtp.tile([128, KU], BF16)
            nc.scalar.dma_start(
                out=vt[:].rearrange("p (k o) -> p k o", k=KU),
                in_=v_in[:, :].rearrange("(k p) o -> p k o", p=128),
            )
            wxall = wtp.tile([128, KIN * 4 * HC], F32R)
            nc.sync.dma_start(
                out=wxall[:].rearrange("p (k u) -> p k u", k=KIN),
                in_=wx[:, :].rearrange("(k p) u -> p k u", p=128),
            )
            whall = wtp.tile([128, KH * 4 * HC], F32R)
            nc.scalar.dma_start(
                out=whall[:].rearrange("p (k u) -> p k u", k=KH),
                in_=wh[:, :].rearrange("(k p) u -> p k u", p=128),
            )
            b_sb = smallp.tile([1, 4 * HC], F32R)
            nc.scalar.dma_start(out=b_sb[:], in_=b_sl[:])

            # ---------------- phase T: input transposes ----------------
            def transpose_to(dst_ap, src_ap, p_in, p_out):
                tr = ptr.tile([128, 128], F32, tag="tr", bufs=2, name="tr")
                nc.tensor.transpose(tr[:p_out, :p_in], src_ap, ident[:p_in, :p_in])
                nc.vector.tensor_copy(dst_ap, tr[:p_out, :p_in])

            htall = smallp.tile([128, KH * B], F32R)
            nc.sync.dma_start(
                out=htall[:].rearrange("p (k b) -> p k b", k=KH),
                in_=h_t[:, :].rearrange("(k p) b -> p k b", p=128),
            )
            htk = [htall[:, B * k : B * (k + 1)] for k in range(KH)]
            hlall = smallp.tile([128, KH * BL], F32R)
            nc.scalar.dma_start(
                out=hlall[:].rearrange("p (k b) -> p k b", k=KH),
                in_=hl_t[:, :].rearrange("(k p) b -> p k b", p=128),
            )
            hlk = [hlall[:, BL * k : BL * (k + 1)] for k in range(KH)]
            xtk = []  # x^T chunks [128, 64] f32r: emb (2) then ctx (8)
            for k in range(KE):
                d = smallp.tile([128, B], F32R, name=f"xtk{k}")
                transpose_to(d[:], emb_sb[:, 128 * k : 128 * (k + 1)], B, 128)
                xtk.append(d)

            # ---------------- early LSTM partial: z += emb@Wx + h@Wh + b ------
            zp = pacc.tile([B, 4 * HC], F32, tag="acc", name="zp")
            for k in range(KE):
                nc.tensor.matmul(
                    zp[:],
                    xtk[k][:],
                    wxall[:, 4 * HC * k : 4 * HC * (k + 1)],
                    start=(k == 0),
                    stop=False,
                )
            for k in range(KH):
                nc.tensor.matmul(
                    zp[:],
                    htk[k],
                    whall[:, 4 * HC * k : 4 * HC * (k + 1)],
                    start=False,
                    stop=False,
                )
            nc.tensor.matmul(
                zp[:], ones_r[0:1, 0:B], b_sb[0:1, :], start=False, stop=False
            )

            # ---------------- phase Q: q = h_loc @ W2 (natural layout) -------
            qp = prow.tile([BL, UNITS], F32, tag="row", name="qp")
            for k in range(KH):
                nc.tensor.matmul(
                    qp[:],
                    hlk[k],
                    w2all[:, UNITS * k : UNITS * (k + 1)],
                    start=(k == 0),
                    stop=(k == KH - 1),
                )
            q_sb = smallp.tile([BL, UNITS], F32)
            nc.vector.tensor_copy(q_sb[:], qp[:])
            qt = smallp.tile([128, KU * BL], F32)  # q^T: [128(u), 8] per u-chunk
            for u in range(KU):
                transpose_to(
                    qt[:, BL * u : BL * (u + 1)],
                    q_sb[:, 128 * u : 128 * (u + 1)],
                    BL,
                    128,
                )

            # ---------------- phase A: attention (bf16, DMA-transposed E) ----
            esums = smallp.tile([1, BL], F32)
            rsum = smallp.tile([1, BL], F32)
            wc8 = smallp.tile([128, BL], BF16)

            etall = attnp.tile([128, KD * BL * S], BF16, tag="big", bufs=1)
            for hf in range(2):
                nc.sync.dma_start(
                    out=etall[:].rearrange("p (k t c) -> p k t c", k=KD, t=2)[
                        :, :, hf, :
                    ],
                    in_=enc_t[:, 512 * hf : 512 * (hf + 1)].rearrange(
                        "(k p) c -> p k c", p=128
                    ),
                )
            etg = [
                [etall[:, 1024 * d + 512 * hf : 1024 * d + 512 * (hf + 1)] for hf in range(2)]
                for d in range(KD)
            ]
            eall = epool.tile([S, BL * DENC], BF16, tag="ebig", bufs=1)
            nc.sync.dma_start(
                out=eall[:].rearrange("s (b d) -> s b d", b=BL),
                in_=enc[:, :, :].rearrange("b s d -> s b d"),
            )
            e_tiles = [eall[:, DENC * b_ : DENC * (b_ + 1)] for b_ in range(BL)]

            for hf in range(2):
                tt = []
                for u in range(KU):
                    pa = pap.tile([128, 512], F32, tag="pa", name=f"pa{u}_{hf}")
                    for k in range(KD):
                        nc.tensor.matmul(
                            pa[:],
                            w1all[:, UNITS * k + 128 * u : UNITS * k + 128 * (u + 1)],
                            etg[k][hf][:],
                            start=(k == 0),
                            stop=(k == KD - 1),
                        )
                    tt_u = attnp.tile(
                        [128, 512], BF16, tag="tt", bufs=KU + 1, name=f"tt{u}_{hf}"
                    )
                    for j in range(4):
                        b_ = 4 * hf + j
                        nc.scalar.activation(
                            tt_u[:, 128 * j : 128 * (j + 1)],
                            pa[:, 128 * j : 128 * (j + 1)],
                            AF.Tanh,
                            bias=qt[:, BL * u + b_ : BL * u + b_ + 1],
                        )
                    tt.append(tt_u)
                srow = prow.tile([1, 512], F32, tag="row", name=f"srow{hf}")
                for u in range(KU):
                    nc.tensor.matmul(
                        srow[:],
                        vt[:, u : u + 1],
                        tt[u][:],
                        start=(u == 0),
                        stop=(u == KU - 1),
                    )
                erow = []
                for j in range(4):
                    b_ = 4 * hf + j
                    er = smallp.tile([1, S], F32, tag="erow", bufs=4, name=f"erow{b_}")
                    nc.scalar.activation(
                        er[:],
                        srow[0:1, 128 * j : 128 * (j + 1)],
                        AF.Exp,
                        accum_out=esums[0:1, b_ : b_ + 1],
                    )
                    erow.append(er)
                nc.vector.reciprocal(
                    rsum[0:1, 4 * hf : 4 * (hf + 1)], esums[0:1, 4 * hf : 4 * (hf + 1)]
                )
                for j in range(4):
                    b_ = 4 * hf + j
                    wrow = smallp.tile([1, S], BF16, tag="wrow", bufs=3, name="wrow")
                    nc.vector.tensor_scalar_mul(
                        wrow[:], erow[j][:], rsum[0:1, b_ : b_ + 1]
                    )
                    pw = ptr.tile([128, 128], F32, tag="tr", bufs=2, name="pw")
                    nc.tensor.matmul(
                        pw[:, 0:1], wrow[:], ones_b[0:1, 0:1], start=True, stop=True
                    )
                    nc.vector.tensor_copy(wc8[:, b_ : b_ + 1], pw[:, 0:1])
                    csb = smallp.tile([1, DENC], F32, tag="csb", bufs=2, name="csb")
                    for n in range(2):
                        crow = prow.tile([1, 512], F32, tag="row", name="crow")
                        nc.tensor.matmul(
                            crow[:],
                            wc8[:, b_ : b_ + 1],
                            eall[:, DENC * b_ + 512 * n : DENC * b_ + 512 * (n + 1)],
                            start=True,
                            stop=True,
                        )
                        nc.vector.tensor_copy(
                            csb[0:1, 512 * n : 512 * (n + 1)], crow[:]
                        )
                    nc.scalar.dma_start(out=ctx_ag_in[b_ : b_ + 1, :], in_=csb[:])

            # ---------------- AllGather context ----------------
            nc.gpsimd.collective_compute(
                "AllGather",
                ALU.bypass,
                replica_groups=rg,
                ins=[ctx_ag_in[:].opt()],
                outs=[ctx_ag_out[:].opt()],
            )
            ctx_sb = smallp.tile([B, DENC], F32, tag="hbuf", bufs=1, name="ctx_sb")
            nc.sync.dma_start(out=ctx_sb[:], in_=ctx_ag_out[:])
            for d in range(KD):
                x = smallp.tile([128, B], F32R, name=f"xtc{d}")
                transpose_to(x[:], ctx_sb[:, 128 * d : 128 * (d + 1)], B, 128)
                xtk.append(x)

            # keep PE warm into the LSTM (keyed on first ctx chunk)
            warml = prow.tile([B, 512], F32, tag="row", name="warml")
            for wk in range(8):
                nc.tensor.matmul(
                    warml[:],
                    xtk[KE][:, 0:B],
                    w2all[:, 512 * (wk % KD) : 512 * (wk % KD + 1)],
                    start=(wk == 0),
                    stop=(wk == 7),
                )
            warml_sb = smallp.tile([1, 1], F32)
            nc.vector.tensor_copy(warml_sb[:], warml[0:1, 0:1])

            # ---------------- phase L: LSTM ctx part ----
            for k in range(KE, KIN):
                nc.tensor.matmul(
                    zp[:],
                    xtk[k][:],
                    wxall[:, 4 * HC * k : 4 * HC * (k + 1)],
                    start=False,
                    stop=(k == KIN - 1),
                )
            gi = smallp.tile([B, HC], F32)
            gf = smallp.tile([B, HC], F32)
            gg = smallp.tile([B, HC], F32)
            go = smallp.tile([B, HC], F32)
            nc.scalar.activation(gi[:], zp[:, 0 * HC : 1 * HC], AF.Sigmoid)
            nc.scalar.activation(gf[:], zp[:, 1 * HC : 2 * HC], AF.Sigmoid)
            nc.scalar.activation(gg[:], zp[:, 2 * HC : 3 * HC], AF.Tanh)
            nc.scalar.activation(go[:], zp[:, 3 * HC : 4 * HC], AF.Sigmoid)
            t1 = smallp.tile([B, HC], F32)
            nc.vector.tensor_mul(t1[:], gf[:], c_sb[:])
            t2 = smallp.tile([B, HC], F32)
            nc.vector.tensor_mul(t2[:], gi[:], gg[:])
            cn = smallp.tile([B, HC], F32)
            nc.vector.tensor_add(cn[:], t1[:], t2[:])
            nc.scalar.dma_start(out=c_out[:, :], in_=cn[:])
            th = smallp.tile([B, HC], F32)
            nc.scalar.activation(th[:], cn[:], AF.Tanh)
            hn_nat = smallp.tile([B, HC], F32)
            nc.vector.tensor_mul(hn_nat[:], go[:], th[:])
            nc.scalar.dma_start(out=h_out[:, :], in_=hn_nat[:])
            hn_r = smallp.tile([128, B], F32R)
            transpose_to(hn_r[:], hn_nat[:, :], B, 128)
            nc.scalar.dma_start(out=hn_ag_in[:, :], in_=hn_r[:HC, :])

            # ---------------- AllGather h_new^T ----------------
            nc.gpsimd.collective_compute(
                "AllGather",
                ALU.bypass,
                replica_groups=rg,
                ins=[hn_ag_in[:].opt()],
                outs=[hn_ag_out[:].opt()],
            )
            # keep the PE HAM-warm through the AllGather gap
            warmp = pacc.tile([B, 512], F32, tag="acc", name="warmp")
            for wk in range(14):
                nc.tensor.matmul(
                    warmp[:],
                    hn_r[:, 0:B],
                    w2all[:, 512 * (wk % KD) : 512 * (wk % KD + 1)],
                    start=(wk == 0),
                    stop=(wk == 13),
                )
            warm_sb = smallp.tile([1, 1], F32)
            nc.vector.tensor_copy(warm_sb[:], warmp[0:1, 0:1])
            hnall_r = smallp.tile([128, KH * B], F32R)
            nc.sync.dma_start(
                out=hnall_r[:].rearrange("p (k b) -> p k b", k=KH),
                in_=hn_ag_out[:, :].rearrange("(k p) b -> p k b", p=128),
            )
            hnall = smallp.tile([128, KH * B], BF16)
            nc.vector.tensor_copy(hnall[:], hnall_r[:])

            # ---------------- phase F: fc + vocab softmax ----------------
            bfc_sb = epool.tile([1, VC], BF16, tag="ebig", bufs=1, name="bfc_sb")
            nc.scalar.dma_start(out=bfc_sb[:], in_=bfc[:])
            sums = []
            for half in range(2):
                sums.append(smallp.tile([B, 4], F32, name=f"sums{half}"))
            expall = attnp.tile([B, VC], F32, tag="big", bufs=1, name="expall")
            for n in range(NNC):
                wp = wfcp.tile([128, KH * NCH], BF16, tag="wfc", name=f"wp{n}")
                nc.sync.dma_start(
                    out=wp[:].rearrange("p (k j) -> p k j", k=KH),
                    in_=wfc[:, NCH * n : NCH * (n + 1)].rearrange(
                        "(k p) j -> p k j", p=128
                    ),
                )
                pf = pap.tile([B, NCH], F32, tag="pa", name=f"pf{n}")
                for k in range(KH):
                    nc.tensor.matmul(
                        pf[:],
                        hnall[:, B * k : B * (k + 1)],
                        wp[:, NCH * k : NCH * (k + 1)],
                        start=(k == 0),
                        stop=False,
                    )
                nc.tensor.matmul(
                    pf[:],
                    ones_b[0:1, 0:B],
                    bfc_sb[0:1, NCH * n : NCH * (n + 1)],
                    start=False,
                    stop=True,
                )
                nc.scalar.activation(
                    expall[:, NCH * n : NCH * (n + 1)],
                    pf[:],
                    AF.Exp,
                    accum_out=sums[n // 4][:, n % 4 : n % 4 + 1],
                )
            tot = smallp.tile([B, 2], F32)
            for half in range(2):
                nc.vector.tensor_reduce(
                    tot[:, half : half + 1], sums[half][:], AX.X, ALU.add
                )
            tsum = smallp.tile([B, 1], F32)
            nc.vector.tensor_add(tsum[:], tot[:, 0:1], tot[:, 1:2])
            nc.scalar.dma_start(out=ar_half[0][:, :], in_=tsum[:])
            nc.gpsimd.collective_compute(
                "AllReduce",
                ALU.add,
                replica_groups=rg,
                ins=[ar_half[0][:].opt()],
                outs=[ar_half_out[0][:].opt()],
            )
            ssum = smallp.tile([B, 1], F32)
            nc.scalar.dma_start(out=ssum[:], in_=ar_half_out[0][:, :])
            rv = smallp.tile([B, 1], F32)
            nc.vector.reciprocal(rv[:], ssum[:])
            for half in range(2):
                sl = slice(2000 * half, 2000 * (half + 1))
                nc.vector.tensor_scalar_mul(
                    expall[:, sl], expall[:, sl], rv[:, 0:1]
                )
                nc.sync.dma_start(out=probs_out[:, sl], in_=expall[:, sl])
    return nc


_NC_CACHE = None


def get_nc():
    global _NC_CACHE
    if _NC_CACHE is None:
        nc = bacc.Bacc(
            "TRN2", target_bir_lowering=False, debug=False, num_devices=N_CORES
        )
        build_kernel(nc)
        nc.compile()
        _NC_CACHE = nc
    return _NC_CACHE


def make_in_maps(token, enc_out, h, c, emb_table, W1, W2, V, Wx, Wh, b, Wfc, bfc):
    token = np.ascontiguousarray(np.asarray(token, dtype=np.int32))
    enc_out = np.asarray(enc_out, dtype=np.float32)
    h = np.asarray(h, dtype=np.float32)
    c = np.asarray(c, dtype=np.float32)
    emb_table = np.ascontiguousarray(np.asarray(emb_table, dtype=np.float32))
    W1 = np.ascontiguousarray(np.asarray(W1, dtype=np.float32))
    W2 = np.ascontiguousarray(np.asarray(W2, dtype=np.float32))
    V = np.ascontiguousarray(np.asarray(V, dtype=np.float32))
    Wx = np.asarray(Wx, dtype=np.float32)
    Wh = np.asarray(Wh, dtype=np.float32)
    b = np.asarray(b, dtype=np.float32)
    Wfc = np.asarray(Wfc, dtype=np.float32)
    bfc = np.asarray(bfc, dtype=np.float32)

    in_maps = []
    for k in range(N_CORES):
        hc = slice(HC * k, HC * (k + 1))
        gate_cols = np.concatenate(
            [np.arange(HC * k, HC * (k + 1)) + j * H for j in range(4)]
        )
        in_maps.append(
            {
                "enc": np.ascontiguousarray(enc_out[BL * k : BL * (k + 1)]).astype(
                    ml_dtypes.bfloat16
                ),
                "enc_t": np.ascontiguousarray(
                    enc_out[BL * k : BL * (k + 1)]
                    .transpose(2, 0, 1)
                    .reshape(DENC, BL * S)
                ).astype(ml_dtypes.bfloat16),
                "hT": np.ascontiguousarray(h.T),
                "h_locT": np.ascontiguousarray(h[BL * k : BL * (k + 1)].T),
                "token": token,
                "emb_table": emb_table,
                "c_sl": np.ascontiguousarray(c[:, hc]),
                "W1": W1.astype(ml_dtypes.bfloat16),
                "W2": W2,
                "V": V.astype(ml_dtypes.bfloat16),
                "Wx_sl": np.ascontiguousarray(Wx[:, gate_cols]),
                "Wh_sl": np.ascontiguousarray(Wh[:, gate_cols]),
                "b_sl": np.ascontiguousarray(b[gate_cols]).reshape(1, 4 * HC),
                "Wfc_sl": np.ascontiguousarray(
                    Wfc[:, VC * k : VC * (k + 1)]
                ).astype(ml_dtypes.bfloat16),
                "bfc_sl": np.ascontiguousarray(bfc[VC * k : VC * (k + 1)])
                .reshape(1, VC)
                .astype(ml_dtypes.bfloat16),
            }
        )
    return in_maps


def assemble(results):
    probs = np.concatenate([results[k]["probs"] for k in range(N_CORES)], axis=1)
    h_new = np.concatenate([results[k]["h_sl"] for k in range(N_CORES)], axis=1)
    c_new = np.concatenate([results[k]["c_sl_out"] for k in range(N_CORES)], axis=1)
    return probs, h_new, c_new


def kernel(token, enc_out, h, c, emb_table, W1, W2, V, Wx, Wh, b, Wfc, bfc):
    nc = get_nc()
    in_maps = make_in_maps(
        token, enc_out, h, c, emb_table, W1, W2, V, Wx, Wh, b, Wfc, bfc
    )
    res = run_bass_kernel_spmd(nc, in_maps, list(range(N_CORES))).results
    return assemble(res)
